# revision 21
# baseline (speedup 1.0000x reference)
"""InfoNCE-style cosine-similarity cross-entropy loss on 8 Trainium2 cores.

Math (matches the reference):
    xn   = x / max(||x_i||, 1e-8)
    sim  = xn @ xn.T                      # [N, N]
    logits = (sim - 1e12 * I) / 0.05
    loss = -mean_i log_softmax(logits)[i, i^1]

Since logits <= 20 (cosine <= 1), plain log(sum(exp)) is stable in fp32 and
no running max is needed.  loss_i = log(sum_j exp(20*sim_ij, diag masked))
                                    - 20*sim[i, i^1].

Sharding: rows are split across 8 cores (1024 rows each).  Every core gets
the full x, but rotated so its OWN 1024 rows come first ("xs" = roll(x,
-1024*c, axis=0)).  The device program is completely core-independent: it
normalizes + transposes all 8192 rows into an SBUF-resident bf16 [768, 8192]
buffer, then computes its [1024, 8192] slice of sim with the stationary
operand taken from columns 0..1023 (its own rows).  The diagonal and the
label pair (i^1) then always fall in a fixed, core-independent position.
Per-row losses are DMA'd out; the host gathers and takes the mean.
"""

import sys

if "/opt/trn_rl_repo" not in sys.path:
    sys.path.insert(0, "/opt/trn_rl_repo")

import numpy as np

import concourse.bacc as bacc
import concourse.bass as bass
import concourse.mybir as mybir
import concourse.tile as tile
from concourse.bass_utils import run_bass_kernel_spmd

N = 8192          # total rows
D = 768           # feature dim
NCORES = 8
R = N // NCORES   # 1024 rows per core
P = 128           # partitions
KC = D // P       # 6 k-chunks of 128
NT = N // P       # 64 row tiles
RB = R // P       # 8 row blocks per core
CBW = 1024        # column block width (two PSUM banks of fp32)
CB = N // CBW     # 8 column blocks
HALF = 512        # one PSUM bank / one DoubleRow matmul output
KP = KC // 2      # 3 double-row k-pair chunks
GRP = 4           # tiles per norm batch (small => first transpose starts sooner)

TEMP_INV = 20.0   # 1 / 0.05
NEG = -1.0e6      # value written on the diagonal before exp (exp(20*NEG)=0)

dt = mybir.dt
AF = mybir.ActivationFunctionType
ALU = mybir.AluOpType


def build_program(debug: bool = False):
    nc = bacc.Bacc("TRN2", target_bir_lowering=False, debug=debug)

    xs = nc.dram_tensor("xs", [N, D], dt.bfloat16, kind="ExternalInput")
    pmask = nc.dram_tensor("pmask", [P, P], dt.float32, kind="ExternalInput")
    emask = nc.dram_tensor("emask", [P, P], dt.uint8, kind="ExternalInput")
    ident = nc.dram_tensor("ident", [P, P], dt.bfloat16, kind="ExternalInput")
    losses_d = nc.dram_tensor("losses", [P, RB], dt.float32, kind="ExternalOutput")

    with tile.TileContext(nc) as tc:
        with (
            tc.tile_pool(name="const", bufs=1) as constp,
            tc.tile_pool(name="xnbuf", bufs=1) as xnp,
            tc.tile_pool(name="ld", bufs=2 * GRP) as ldp,
            tc.tile_pool(name="sq", bufs=3) as sqp,
            tc.tile_pool(name="nb", bufs=3) as nbp,
            tc.tile_pool(name="small", bufs=4) as smallp,
            tc.tile_pool(name="tps", bufs=2, space=bass.MemorySpace.PSUM) as tpsp,
            tc.tile_pool(name="acc", bufs=3, space=bass.MemorySpace.PSUM) as accp,
            tc.tile_pool(name="et", bufs=3) as etp,
            tc.tile_pool(name="res", bufs=1) as resp,
        ):
            pm = constp.tile([P, P], dt.float32, tag="pm")
            nc.sync.dma_start(pm[:], pmask[:])
            em = constp.tile([P, P], dt.uint8, tag="em")
            nc.sync.dma_start(em[:], emask[:])
            idt = constp.tile([P, P], dt.bfloat16, tag="idt")
            nc.sync.dma_start(idt[:], ident[:])
            negt = constp.tile([P, P], dt.float32, tag="negt")
            nc.vector.memset(negt[:], NEG)

            # xn transposed: [feature%128, k-chunk, row]  (fp8e4, SBUF-resident)
            xnT = xnp.tile([P, KC, N], dt.float8e4, tag="xnT")

            loss_sb = resp.tile([P, RB], dt.float32, tag="loss")
            stots = resp.tile([P, RB], dt.float32, tag="stots")
            labs = resp.tile([P, RB], dt.float32, tag="labs")

            # ---------------- Phase A: normalize + transpose ----------------
            for g in range(NT // GRP):
                ssg = smallp.tile([P, GRP], dt.float32, tag="ssg")
                xts = []
                for j in range(GRP):
                    t = g * GRP + j
                    xt = ldp.tile([P, D], dt.bfloat16, tag="xt")
                    nc.sync.dma_start(xt[:], xs[P * t : P * (t + 1), :])
                    sq = sqp.tile([P, D], dt.float32, tag="sq")
                    nc.scalar.activation(
                        sq[:], xt[:], AF.Square, accum_out=ssg[:, j : j + 1]
                    )
                    xts.append(xt)
                nrm = smallp.tile([P, GRP], dt.float32, tag="nrm")
                nc.scalar.sqrt(nrm[:], ssg[:])
                nc.vector.tensor_scalar_max(nrm[:], nrm[:], 1e-8)
                rn = smallp.tile([P, GRP], dt.float32, tag="rn")
                nc.vector.reciprocal(rn[:], nrm[:])
                for j in range(GRP):
                    t = g * GRP + j
                    xnb = nbp.tile([P, D], dt.bfloat16, tag="xnb")
                    nc.vector.tensor_scalar_mul(xnb[:], xts[j][:], rn[:, j : j + 1])
                    pt = tpsp.tile([P, KC * P], dt.bfloat16, tag="pt")
                    for kc in range(KC):
                        nc.tensor.transpose(
                            pt[:, P * kc : P * (kc + 1)],
                            xnb[:, P * kc : P * (kc + 1)],
                            idt[:],
                        )
                    src = pt[:].rearrange("p (k c) -> p k c", k=KC)
                    dst = xnT[:, :, P * t : P * (t + 1)]
                    nc.vector.tensor_copy(dst, src)

            # ---------------- Phase B: sim block + online CE ----------------
            for rb in range(RB):
                sums = smallp.tile([P, CB], dt.float32, tag="sums")
                # own rows are columns 0..R-1, so rb's diagonal block always
                # lands in cb == 0 at offset 128*rb
                for q in range(CB):
                    ps = accp.tile([P, CBW], dt.float32, tag="ps")
                    for h in range(CBW // HALF):
                        for kp in range(KP):
                            nc.tensor.matmul(
                                ps[:, HALF * h : HALF * (h + 1)],
                                xnT[:, 2 * kp : 2 * kp + 2, P * rb : P * (rb + 1)],
                                xnT[
                                    :,
                                    2 * kp : 2 * kp + 2,
                                    CBW * q + HALF * h : CBW * q + HALF * (h + 1),
                                ],
                                start=(kp == 0),
                                stop=(kp == KP - 1),
                                perf_mode=mybir.MatmulPerfMode.DoubleRow,
                            )
                    if q == 0:
                        blk = ps[:, P * rb : P * (rb + 1)]
                        scr = etp.tile([P, P], dt.float32, tag="scr")
                        # label logit: sim[i, i^1] via perm-mask multiply+reduce
                        nc.vector.tensor_tensor(scr[:], blk, pm[:], op=ALU.mult)
                        nc.vector.tensor_reduce(
                            labs[:, rb : rb + 1],
                            scr[:],
                            axis=mybir.AxisListType.X,
                            op=ALU.add,
                        )
                        # mask the diagonal
                        nc.vector.copy_predicated(blk, em[:], negt[:])
                    et = etp.tile([P, CBW], dt.float32, tag="et")
                    nc.scalar.activation(
                        et[:],
                        ps[:],
                        AF.Exp,
                        scale=TEMP_INV,
                        accum_out=sums[:, q : q + 1],
                    )
                nc.vector.tensor_reduce(
                    stots[:, rb : rb + 1],
                    sums[:],
                    axis=mybir.AxisListType.X,
                    op=ALU.add,
                )
            # batched epilogue: one Ln over all row blocks, then combine
            lnt = resp.tile([P, RB], dt.float32, tag="lnt")
            nc.scalar.activation(lnt[:], stots[:], AF.Ln)
            nc.vector.tensor_scalar(
                out=loss_sb[:],
                in0=labs[:],
                scalar1=-TEMP_INV,
                scalar2=None,
                op0=ALU.mult,
            )
            nc.vector.tensor_tensor(
                out=loss_sb[:], in0=loss_sb[:], in1=lnt[:], op=ALU.add
            )

            nc.sync.dma_start(losses_d[:], loss_sb[:])

    nc.compile()
    return nc


_CACHE = {}


def _get_program(debug: bool = False):
    key = bool(debug)
    if key not in _CACHE:
        _CACHE[key] = build_program(debug=debug)
    return _CACHE[key]


def _host_inputs(x: np.ndarray):
    import ml_dtypes

    pmask = np.zeros((P, P), dtype=np.float32)
    idx = np.arange(P)
    pmask[idx, idx ^ 1] = 1.0
    emask = np.eye(P, dtype=np.uint8)
    ident = np.eye(P, dtype=np.float32).astype(ml_dtypes.bfloat16)
    consts = {"pmask": pmask, "emask": emask, "ident": ident}
    in_maps = []
    for c in range(NCORES):
        xs = np.roll(x, -R * c, axis=0).astype(ml_dtypes.bfloat16)
        in_maps.append({"xs": np.ascontiguousarray(xs), **consts})
    return in_maps


def _run(x: np.ndarray, trace: bool = False):
    nc = _get_program(debug=False)
    in_maps = _host_inputs(np.asarray(x))
    res = run_bass_kernel_spmd(nc, in_maps, list(range(NCORES)), trace=trace)
    # losses[c][p, rb] is the loss of global row 1024*c + 128*rb + p; the
    # final reduction is a plain mean so layout does not matter.
    all_losses = np.stack([res.results[c]["losses"] for c in range(NCORES)])
    loss = np.float32(np.mean(all_losses.astype(np.float64)))
    return loss, res


def kernel(x: np.ndarray) -> np.ndarray:
    loss, _ = _run(x, trace=False)
    return np.asarray(loss, dtype=np.float32)


# revision 22
# speedup vs baseline: 1.0723x; 1.0723x over previous
"""InfoNCE-style cosine-similarity cross-entropy loss on 8 Trainium2 cores.

Math (matches the reference):
    xn   = x / max(||x_i||, 1e-8)
    sim  = xn @ xn.T                      # [N, N]
    logits = (sim - 1e12 * I) / 0.05
    loss = -mean_i log_softmax(logits)[i, i^1]

Since logits <= 20 (cosine <= 1), plain log(sum(exp)) is stable in fp32 and
no running max is needed.  loss_i = log(sum_j exp(20*sim_ij, diag masked))
                                    - 20*sim[i, i^1].

Sharding: rows are split across 8 cores (1024 rows each).  Every core gets
the full x, but rotated so its OWN 1024 rows come first ("xs" = roll(x,
-1024*c, axis=0)).  The device program is completely core-independent: it
normalizes + transposes all 8192 rows into an SBUF-resident bf16 [768, 8192]
buffer, then computes its [1024, 8192] slice of sim with the stationary
operand taken from columns 0..1023 (its own rows).  The diagonal and the
label pair (i^1) then always fall in a fixed, core-independent position.
Per-row losses are DMA'd out; the host gathers and takes the mean.
"""

import sys

if "/opt/trn_rl_repo" not in sys.path:
    sys.path.insert(0, "/opt/trn_rl_repo")

import numpy as np

import concourse.bacc as bacc
import concourse.bass as bass
import concourse.mybir as mybir
import concourse.tile as tile
from concourse.bass_utils import run_bass_kernel_spmd

N = 8192          # total rows
D = 768           # feature dim
NCORES = 8
R = N // NCORES   # 1024 rows per core
P = 128           # partitions
KC = D // P       # 6 k-chunks of 128
NT = N // P       # 64 row tiles
RB = R // P       # 8 row blocks per core
CBW = 1024        # column block width (two PSUM banks of fp32)
CB = N // CBW     # 8 column blocks
HALF = 512        # one PSUM bank / one DoubleRow matmul output
KP = KC // 2      # 3 double-row k-pair chunks
GRP = 8           # tiles per norm batch

TEMP_INV = 20.0   # 1 / 0.05
NEG = -1.0e6      # value written on the diagonal before exp (exp(20*NEG)=0)

dt = mybir.dt
AF = mybir.ActivationFunctionType
ALU = mybir.AluOpType


def build_program(debug: bool = False):
    nc = bacc.Bacc("TRN2", target_bir_lowering=False, debug=debug)

    xs = nc.dram_tensor("xs", [N, D], dt.bfloat16, kind="ExternalInput")
    pmask = nc.dram_tensor("pmask", [P, P], dt.float32, kind="ExternalInput")
    emask = nc.dram_tensor("emask", [P, P], dt.uint8, kind="ExternalInput")
    ident = nc.dram_tensor("ident", [P, P], dt.bfloat16, kind="ExternalInput")
    losses_d = nc.dram_tensor("losses", [P, RB], dt.float32, kind="ExternalOutput")

    with tile.TileContext(nc) as tc:
        with (
            tc.tile_pool(name="const", bufs=1) as constp,
            tc.tile_pool(name="xnbuf", bufs=1) as xnp,
            tc.tile_pool(name="ld", bufs=2 * GRP) as ldp,
            tc.tile_pool(name="sq", bufs=3) as sqp,
            tc.tile_pool(name="nb", bufs=3) as nbp,
            tc.tile_pool(name="small", bufs=4) as smallp,
            tc.tile_pool(name="tps", bufs=2, space=bass.MemorySpace.PSUM) as tpsp,
            tc.tile_pool(name="acc", bufs=3, space=bass.MemorySpace.PSUM) as accp,
            tc.tile_pool(name="et", bufs=3) as etp,
            tc.tile_pool(name="res", bufs=1) as resp,
        ):
            pm = constp.tile([P, P], dt.float32, tag="pm")
            nc.sync.dma_start(pm[:], pmask[:])
            em = constp.tile([P, P], dt.uint8, tag="em")
            nc.sync.dma_start(em[:], emask[:])
            idt = constp.tile([P, P], dt.bfloat16, tag="idt")
            nc.sync.dma_start(idt[:], ident[:])
            negt = constp.tile([P, P], dt.float32, tag="negt")
            nc.vector.memset(negt[:], NEG)

            # xn transposed: [feature%128, k-chunk, row]  (fp8e4, SBUF-resident)
            xnT = xnp.tile([P, KC, N], dt.float8e4, tag="xnT")

            loss_sb = resp.tile([P, RB], dt.float32, tag="loss")
            stots = resp.tile([P, RB], dt.float32, tag="stots")
            labs = resp.tile([P, RB], dt.float32, tag="labs")

            # ---------------- Phase A: normalize + transpose ----------------
            for g in range(NT // GRP):
                ssg = smallp.tile([P, GRP], dt.float32, tag="ssg")
                xts = []
                for j in range(GRP):
                    t = g * GRP + j
                    xt = ldp.tile([P, D], dt.bfloat16, tag="xt")
                    nc.sync.dma_start(xt[:], xs[P * t : P * (t + 1), :])
                    sq = sqp.tile([P, D], dt.float32, tag="sq")
                    nc.scalar.activation(
                        sq[:], xt[:], AF.Square, accum_out=ssg[:, j : j + 1]
                    )
                    xts.append(xt)
                nrm = smallp.tile([P, GRP], dt.float32, tag="nrm")
                nc.scalar.sqrt(nrm[:], ssg[:])
                nc.vector.tensor_scalar_max(nrm[:], nrm[:], 1e-8)
                rn = smallp.tile([P, GRP], dt.float32, tag="rn")
                nc.vector.reciprocal(rn[:], nrm[:])
                for j in range(GRP):
                    t = g * GRP + j
                    xnb = nbp.tile([P, D], dt.bfloat16, tag="xnb")
                    nc.vector.tensor_scalar_mul(xnb[:], xts[j][:], rn[:, j : j + 1])
                    pt = tpsp.tile([P, KC * P], dt.bfloat16, tag="pt")
                    for kc in range(KC):
                        nc.tensor.transpose(
                            pt[:, P * kc : P * (kc + 1)],
                            xnb[:, P * kc : P * (kc + 1)],
                            idt[:],
                        )
                    src = pt[:].rearrange("p (k c) -> p k c", k=KC)
                    dst = xnT[:, :, P * t : P * (t + 1)]
                    nc.vector.tensor_copy(dst, src)

            # ---------------- Phase B: sim block + online CE ----------------
            for rb in range(RB):
                sums = smallp.tile([P, CB], dt.float32, tag="sums")
                # own rows are columns 0..R-1, so rb's diagonal block always
                # lands in cb == 0 at offset 128*rb
                for q in range(CB):
                    ps = accp.tile([P, CBW], dt.float32, tag="ps")
                    for h in range(CBW // HALF):
                        for kp in range(KP):
                            nc.tensor.matmul(
                                ps[:, HALF * h : HALF * (h + 1)],
                                xnT[:, 2 * kp : 2 * kp + 2, P * rb : P * (rb + 1)],
                                xnT[
                                    :,
                                    2 * kp : 2 * kp + 2,
                                    CBW * q + HALF * h : CBW * q + HALF * (h + 1),
                                ],
                                start=(kp == 0),
                                stop=(kp == KP - 1),
                                perf_mode=mybir.MatmulPerfMode.DoubleRow,
                            )
                    if q == 0:
                        blk = ps[:, P * rb : P * (rb + 1)]
                        scr = etp.tile([P, P], dt.float32, tag="scr")
                        # label logit: sim[i, i^1] via perm-mask multiply+reduce
                        nc.vector.tensor_tensor(scr[:], blk, pm[:], op=ALU.mult)
                        nc.vector.tensor_reduce(
                            labs[:, rb : rb + 1],
                            scr[:],
                            axis=mybir.AxisListType.X,
                            op=ALU.add,
                        )
                        # mask the diagonal
                        nc.vector.copy_predicated(blk, em[:], negt[:])
                    et = etp.tile([P, CBW], dt.float32, tag="et")
                    nc.scalar.activation(
                        et[:],
                        ps[:],
                        AF.Exp,
                        scale=TEMP_INV,
                        accum_out=sums[:, q : q + 1],
                    )
                nc.vector.tensor_reduce(
                    stots[:, rb : rb + 1],
                    sums[:],
                    axis=mybir.AxisListType.X,
                    op=ALU.add,
                )
            # batched epilogue: one Ln over all row blocks, then combine
            lnt = resp.tile([P, RB], dt.float32, tag="lnt")
            nc.scalar.activation(lnt[:], stots[:], AF.Ln)
            nc.vector.tensor_scalar(
                out=loss_sb[:],
                in0=labs[:],
                scalar1=-TEMP_INV,
                scalar2=None,
                op0=ALU.mult,
            )
            nc.vector.tensor_tensor(
                out=loss_sb[:], in0=loss_sb[:], in1=lnt[:], op=ALU.add
            )

            nc.sync.dma_start(losses_d[:], loss_sb[:])

    nc.compile()
    return nc


_CACHE = {}


def _get_program(debug: bool = False):
    key = bool(debug)
    if key not in _CACHE:
        _CACHE[key] = build_program(debug=debug)
    return _CACHE[key]


def _host_inputs(x: np.ndarray):
    import ml_dtypes

    pmask = np.zeros((P, P), dtype=np.float32)
    idx = np.arange(P)
    pmask[idx, idx ^ 1] = 1.0
    emask = np.eye(P, dtype=np.uint8)
    ident = np.eye(P, dtype=np.float32).astype(ml_dtypes.bfloat16)
    consts = {"pmask": pmask, "emask": emask, "ident": ident}
    in_maps = []
    for c in range(NCORES):
        xs = np.roll(x, -R * c, axis=0).astype(ml_dtypes.bfloat16)
        in_maps.append({"xs": np.ascontiguousarray(xs), **consts})
    return in_maps


def _run(x: np.ndarray, trace: bool = False):
    nc = _get_program(debug=False)
    in_maps = _host_inputs(np.asarray(x))
    res = run_bass_kernel_spmd(nc, in_maps, list(range(NCORES)), trace=trace)
    # losses[c][p, rb] is the loss of global row 1024*c + 128*rb + p; the
    # final reduction is a plain mean so layout does not matter.
    all_losses = np.stack([res.results[c]["losses"] for c in range(NCORES)])
    loss = np.float32(np.mean(all_losses.astype(np.float64)))
    return loss, res


def kernel(x: np.ndarray) -> np.ndarray:
    loss, _ = _run(x, trace=False)
    return np.asarray(loss, dtype=np.float32)
